# revision 25
# baseline (speedup 1.0000x reference)
"""CrossAttentionQuerySelector Trainium2 kernel (8-core data parallel).

Math (per sample row n of N = B*T, with K=7 candidates, D=512, H=8 heads,
S=3 slots):
  kv      = cand reshaped [N, K, D]
  scores  = (slotq proj) . (kv @ Wk^T) / sqrt(hd)      -> folded: kv @ Qs^T
  attn    = softmax over K
  o       = attn-combine of vh = kv @ Wv^T             (head-blocked)
  attn_out= o @ out_w^T (+ out_w@bv + out_b)
  q       = LN1(slot_q + attn_out) (+ b1n) + slot_se
  y       = LN2(q + gelu(q@w1^T)@w2^T (+biases))

Device mapping per core (2048 samples, supertiles of 512), v2:
  - everything on the matmul path is bf16 (PSUM accumulation stays f32);
    host pre-transposes cand to kvT [512, 14336] bf16 and permutes Wv/out_w
    into the e' = hd*8+h basis so the attn expansion pattern is uniform per
    partition (p % 8 == h).
  - phase A: scores^T [24,448] per 448-row tile; exp on ACT; z-reduce +
    normalize on Pool; attn (bf16) bounced to DRAM and re-loaded with a
    broadcasting AP to [128,448] per slot on the Pool DMA queue; vh kept
    bf16 in SBUF (ACT PSUM->SBUF copy); combine = DVE bf16 2x multiply
    (one inst per slot over all 4 e-chunks) + grouped reduce split
    DVE (slots 0,1) / Pool (slot 2) -> oT [d',n] bf16.
  - phase B: out-proj / FF per (slot, 512-sample block); slot_q bias and
    (se+b2f) biases folded in as K=1 ones-row matmuls; w1@se folded into
    the gelu per-partition bias (D-major), so the qT evacuation is a single
    unbiased ACT copy of 4 transposes batched in one PSUM bank; LN via
    bn_stats, rstd = exp(-ln/2) on ACT (shared table set with softmax exp),
    LN apply on ACT as Identity(x*rstd + (-mean*rstd)).
"""

import os
import sys

for _p in ("/opt/trn_rl_repo", "/root/.axon_site/_ro/trn_rl_repo"):
    if os.path.isdir(_p) and _p not in sys.path:
        sys.path.insert(0, _p)

import numpy as np
from contextlib import ExitStack

import concourse.bass as bass
import concourse.tile as tile
from concourse import mybir, bacc
from concourse.bass_utils import run_bass_kernel_spmd
from concourse.masks import make_identity

# The activation-table chooser greedily serves Exp from `exp_and_others` and
# Ln from `natural_log`, which forces two table reloads per LayerNorm (we
# compute rstd = exp(-ln(var+eps)/2)). Steer both onto the combined
# `natural_log_exp_and_others` set by hiding Exp/Ln in every other set in the
# list handed to the load-insertion pass (set ids / real act_info.json are
# untouched — a load of the combined set genuinely serves both funcs).
_orig_get_tables = bacc.get_activation_tables


def _patched_get_tables(arch):
    tabs = _orig_get_tables(arch)
    out = {}
    for name, funcs in tabs.items():
        if name != "natural_log_exp_and_others":
            funcs = funcs - {mybir.ActivationFunctionType.Exp,
                             mybir.ActivationFunctionType.Ln}
        out[name] = funcs
    return out


bacc.get_activation_tables = _patched_get_tables

F32 = mybir.dt.float32
BF16 = mybir.dt.bfloat16
AX = mybir.AxisListType
ALU = mybir.AluOpType
ACT_F = mybir.ActivationFunctionType

D = 512
H = 8
HD = 64
S = 3
K = 7
B = 8
T = 2048
NCORES = 8
NSAMP = T                      # samples per core
ROWS = NSAMP * K               # kv rows per core (14336)
ST_SAMP = 512                  # samples per supertile
N_ST = NSAMP // ST_SAMP        # 4 supertiles
A_ROWS = 448                   # rows per phase-A tile (64 samples)
A_SAMP = A_ROWS // K           # 64
N_A = ST_SAMP // A_SAMP        # 8 phase-A tiles per supertile
NC4 = ST_SAMP // 128           # 4 n-chunks per phase-B block
EPS = 1e-5

_CACHE = {}


def _build(has_g1, has_g2, has_b2n, reps=1, comb_bf16=True):
    nc = bacc.Bacc("TRN2", target_bir_lowering=False, debug=False,
                   num_devices=NCORES)

    kvT_d = nc.dram_tensor("kvT", [D, ROWS], BF16, kind="ExternalInput")
    qsT_d = nc.dram_tensor("qsT", [D, 24], BF16, kind="ExternalInput")
    wvT_d = nc.dram_tensor("wvT", [D, D], BF16, kind="ExternalInput")
    owT_d = nc.dram_tensor("owT", [D, D], BF16, kind="ExternalInput")
    w1T_d = nc.dram_tensor("w1T", [D, 2 * D], BF16, kind="ExternalInput")
    w2T_d = nc.dram_tensor("w2T", [2 * D, D], BF16, kind="ExternalInput")
    xbr_d = nc.dram_tensor("xbr", [1, S * D], BF16, kind="ExternalInput")
    ser_d = nc.dram_tensor("ser", [1, S * D], BF16, kind="ExternalInput")
    w1se_d = nc.dram_tensor("w1se", [2 * D, S], F32, kind="ExternalInput")
    one_d = nc.dram_tensor("one", [1, 128], BF16, kind="ExternalInput")
    g1_d = g2_d = b2n_d = None
    if has_g1:
        g1_d = nc.dram_tensor("g1v", [D], F32, kind="ExternalInput")
    if has_g2:
        g2_d = nc.dram_tensor("g2v", [D], F32, kind="ExternalInput")
    if has_b2n:
        b2n_d = nc.dram_tensor("b2nv", [D], F32, kind="ExternalInput")
    out_d = nc.dram_tensor("out", [NSAMP, S, D], F32, kind="ExternalOutput")

    with tile.TileContext(nc) as tc, ExitStack() as ctx:
        cp = ctx.enter_context(tc.tile_pool(name="consts", bufs=1))
        kvp = ctx.enter_context(tc.tile_pool(name="kvp", bufs=3))
        attp = ctx.enter_context(tc.tile_pool(name="attp", bufs=2))
        zp = ctx.enter_context(tc.tile_pool(name="zp", bufs=2))
        vhp = ctx.enter_context(tc.tile_pool(name="vhp", bufs=2))
        axp = ctx.enter_context(tc.tile_pool(name="axp", bufs=3))
        tmpp = ctx.enter_context(tc.tile_pool(name="tmpp", bufs=2))
        otp = ctx.enter_context(tc.tile_pool(name="otp", bufs=2))
        tp = ctx.enter_context(tc.tile_pool(name="tp", bufs=3))
        qtp = ctx.enter_context(tc.tile_pool(name="qtp", bufs=2))
        ff1p = ctx.enter_context(tc.tile_pool(name="ff1p", bufs=2))
        yp = ctx.enter_context(tc.tile_pool(name="yp", bufs=3))
        sp = ctx.enter_context(tc.tile_pool(name="smalls", bufs=6))
        pp = ctx.enter_context(tc.tile_pool(name="psum", bufs=6, space="PSUM"))
        dp = ctx.enter_context(tc.tile_pool(name="dram", bufs=3, space="DRAM"))

        # ---- constants into SBUF ----
        qsT = cp.tile([128, 4, 24], BF16, name="qsT")
        wvT = cp.tile([128, 4, D], BF16, name="wvT")
        owT = cp.tile([128, 4, D], BF16, name="owT")
        w1T = cp.tile([128, 4, 2 * D], BF16, name="w1T")
        w2T = cp.tile([128, 8, D], BF16, name="w2T")
        for k in range(4):
            nc.sync.dma_start(out=qsT[:, k, :], in_=qsT_d[128 * k:128 * (k + 1), :])
            nc.sync.dma_start(out=wvT[:, k, :], in_=wvT_d[128 * k:128 * (k + 1), :])
            nc.scalar.dma_start(out=owT[:, k, :], in_=owT_d[128 * k:128 * (k + 1), :])
            nc.vector.dma_start(out=w1T[:, k, :], in_=w1T_d[128 * k:128 * (k + 1), :])
        for k in range(8):
            nc.vector.dma_start(out=w2T[:, k, :], in_=w2T_d[128 * k:128 * (k + 1), :])
        xbr = cp.tile([1, S, D], BF16, name="xbr")
        nc.scalar.dma_start(out=xbr, in_=xbr_d.ap().rearrange("p (s n) -> p s n", s=S))
        ser = cp.tile([1, S, D], BF16, name="ser")
        nc.scalar.dma_start(out=ser, in_=ser_d.ap().rearrange("p (s n) -> p s n", s=S))
        w1se = cp.tile([128, 8, S], F32, name="w1se")
        for k in range(8):
            nc.vector.dma_start(out=w1se[:, k, :], in_=w1se_d[128 * k:128 * (k + 1), :])
        ones1 = cp.tile([1, 128], BF16, name="ones1")
        nc.scalar.dma_start(out=ones1, in_=one_d[:])
        ident = cp.tile([128, 128], BF16, name="ident")
        make_identity(nc, ident)
        epsb = cp.tile([128, 1], F32, name="epsb")
        nc.vector.memset(epsb, EPS)
        g1b = g2b = b2nb = None
        if has_g1:
            g1b = cp.tile([128, D], F32, name="g1b")
            nc.gpsimd.dma_start(out=g1b, in_=bass.AP(
                tensor=g1_d, offset=0, ap=[[0, 128], [1, D]]))
        if has_g2:
            g2b = cp.tile([128, D], F32, name="g2b")
            nc.gpsimd.dma_start(out=g2b, in_=bass.AP(
                tensor=g2_d, offset=0, ap=[[0, 128], [1, D]]))
        if has_b2n:
            b2nb = cp.tile([128, D], F32, name="b2nb")
            nc.gpsimd.dma_start(out=b2nb, in_=bass.AP(
                tensor=b2n_d, offset=0, ap=[[0, 128], [1, D]]))

        def ln_scale_bias(x_ap, tag):
            """LN stats of x_ap ([128,512]) -> (rstd, nmr) [128,1] tiles.

            rstd = exp(-0.5 * ln(var + eps)) -- Ln and Exp share one ACT
            table set. nmr = -mean * rstd, so the LN apply is a single
            ACT Identity(x * rstd + nmr).
            """
            st6 = sp.tile([128, 6], F32, name=f"st6_{tag}", tag="st6")
            nc.vector.bn_stats(out=st6, in_=x_ap)
            mv = sp.tile([128, 2], F32, name=f"mv_{tag}", tag="mv")
            nc.vector.bn_aggr(out=mv, in_=st6)
            lnv = sp.tile([128, 1], F32, name=f"lnv_{tag}", tag="lnv")
            nc.scalar.activation(lnv, mv[:, 1:2], ACT_F.Ln, bias=epsb[:, 0:1])
            rstd = sp.tile([128, 1], F32, name=f"rstd_{tag}", tag="rstd")
            nc.scalar.activation(rstd, lnv, ACT_F.Exp, scale=-0.5)
            nmr = sp.tile([128, 1], F32, name=f"nmr_{tag}", tag="nmr")
            nc.gpsimd.tensor_scalar(
                out=nmr, in0=mv[:, 0:1], scalar1=rstd[:, 0:1], scalar2=-1.0,
                op0=ALU.mult, op1=ALU.mult)
            return rstd, nmr

        # ---------------- phase emitters ----------------
        def emit_a_tile(st, a, oT):
            nb = st * ST_SAMP
            if True:
                r0 = nb * K + a * A_ROWS
                kv = kvp.tile([128, 4, A_ROWS], BF16, name=f"kv_{st}_{a}", tag="kv")
                nc.sync.dma_start(
                    out=kv,
                    in_=bass.AP(tensor=kvT_d, offset=r0,
                                ap=[[ROWS, 128], [128 * ROWS, 4], [1, A_ROWS]]))

                sc_ps = pp.tile([24, A_ROWS], F32, name=f"sc_{st}_{a}", tag="ps2", bufs=2)
                for k in range(4):
                    nc.tensor.matmul(sc_ps, qsT[:, k, :], kv[:, k, :],
                                     start=(k == 0), stop=(k == 3))
                attn_e = attp.tile([24, A_ROWS], F32, name=f"ate_{st}_{a}",
                                   tag="attn_e")
                nc.scalar.activation(attn_e, sc_ps, ACT_F.Exp)
                z = zp.tile([24, A_SAMP], F32, name=f"z_{st}_{a}", tag="z")
                nc.vector.tensor_reduce(
                    z, attn_e.rearrange("p (n k) -> p n k", k=K),
                    axis=AX.X, op=ALU.add)
                rz = zp.tile([24, A_SAMP], F32, name=f"rz_{st}_{a}", tag="rz")
                nc.vector.reciprocal(rz, z)
                attn = attp.tile([24, A_ROWS], BF16, name=f"att_{st}_{a}",
                                 tag="attn")
                rz_b = bass.AP(tensor=rz.tensor, offset=rz.offset,
                               ap=[list(rz.ap[0]), list(rz.ap[1]), [0, K]])
                nc.gpsimd.tensor_tensor(
                    out=attn.rearrange("p (n k) -> p n k", k=K),
                    in0=attn_e.rearrange("p (n k) -> p n k", k=K),
                    in1=rz_b, op=ALU.mult)
                # bounce attn to DRAM; reload per-slot with broadcast pattern
                attn_dr = dp.tile([24, A_ROWS], BF16, name=f"attd_{st}_{a}",
                                  tag="attn_dr")
                nc.sync.dma_start(out=attn_dr, in_=attn)

                vh = vhp.tile([128, 4, A_ROWS], BF16, name=f"vh_{st}_{a}", tag="vh")
                for e in range(4):
                    vh_ps = pp.tile([128, A_ROWS], F32,
                                    name=f"vhp_{st}_{a}_{e}", tag="ps")
                    for k in range(4):
                        nc.tensor.matmul(vh_ps, wvT[:, k, 128 * e:128 * (e + 1)],
                                         kv[:, k, :],
                                         start=(k == 0), stop=(k == 3))
                    nc.scalar.copy(vh[:, e, :], vh_ps)

                for s in range(S):
                    ax = axp.tile([128, A_ROWS], BF16, name=f"ax_{st}_{a}_{s}",
                                  tag="ax")
                    # attnX[p, :] = attn[(p%8)*3+s, :] — uniform per chunk
                    nc.sync.dma_start(
                        out=ax,
                        in_=bass.AP(tensor=attn_dr.tensor,
                                    offset=attn_dr.offset + s * A_ROWS,
                                    ap=[[0, 16], [3 * A_ROWS, 8], [1, A_ROWS]]))
                    tm = tmpp.tile([128, 4, A_ROWS], BF16,
                                   name=f"tm_{st}_{a}_{s}", tag="tm")
                    ax_b = bass.AP(tensor=ax.tensor, offset=ax.offset,
                                   ap=[list(ax.ap[0]), [0, 4], list(ax.ap[1])])
                    meng = nc.gpsimd if s == 0 else nc.vector
                    meng.tensor_tensor(tm, vh, ax_b, op=ALU.mult)
                    with nc.allow_low_precision(reason="bf16 combine"):
                        nc.vector.tensor_reduce(
                            oT[:, :, s, a * A_SAMP:(a + 1) * A_SAMP],
                            tm.rearrange("p e (n k) -> p e n k", k=K),
                            axis=AX.X, op=ALU.add)

        def emit_b1(st, s, oT):
            """Out-proj + LN1 -> t_sb for one slot (PE then DVE/ACT tail)."""
            t_sb = tp.tile([128, NC4, D], BF16, name=f"t_{st}_{s}", tag="t")
            for c in range(NC4):
                ao_ps = pp.tile([128, D], F32, name=f"ao_{st}_{s}_{c}",
                                tag="ps")
                for k in range(4):
                    nc.tensor.matmul(
                        ao_ps, oT[:, k, s, c * 128:(c + 1) * 128],
                        owT[:, k, :], start=(k == 0), stop=False)
                nc.tensor.matmul(ao_ps, ones1, xbr[:, s, :],
                                 start=False, stop=True)
                rstd, nmr = ln_scale_bias(ao_ps, f"1_{st}_{s}_{c}")
                nc.scalar.activation(
                    t_sb[:, c, :], ao_ps, ACT_F.Identity,
                    scale=rstd[:, 0:1], bias=nmr[:, 0:1])
                if has_g1:
                    nc.vector.tensor_mul(t_sb[:, c, :], t_sb[:, c, :], g1b)
            return t_sb

        def emit_b2(st, s, t_sb):
            nb = st * ST_SAMP
            if True:
                qT = qtp.tile([128, 4, ST_SAMP], BF16, name=f"qT_{st}_{s}",
                              tag="qT")
                for c in range(NC4):
                    tr_ps = pp.tile([128, 4, 128], BF16,
                                    name=f"tr_{st}_{s}_{c}", tag="ps2", bufs=2)
                    for k in range(4):
                        nc.tensor.transpose(
                            tr_ps[:, k, :], t_sb[:, c, 128 * k:128 * (k + 1)],
                            ident)
                    nc.scalar.copy(qT[:, :, c * 128:(c + 1) * 128], tr_ps)

                ff1 = ff1p.tile([128, 8, ST_SAMP], BF16, name=f"ff1_{st}_{s}",
                                tag="ff1")
                for f in range(8):
                    f1_ps = pp.tile([128, ST_SAMP], F32,
                                    name=f"f1_{st}_{s}_{f}", tag="ps")
                    for k in range(4):
                        nc.tensor.matmul(f1_ps, w1T[:, k, 128 * f:128 * (f + 1)],
                                         qT[:, k, :],
                                         start=(k == 0), stop=(k == 3))
                    nc.scalar.activation(ff1[:, f, :], f1_ps, ACT_F.Gelu,
                                         bias=w1se[:, f, s:s + 1])

                for c in range(NC4):
                    f2_ps = pp.tile([128, D], F32, name=f"f2_{st}_{s}_{c}",
                                    tag="ps")
                    for f in range(8):
                        nc.tensor.matmul(f2_ps, ff1[:, f, c * 128:(c + 1) * 128],
                                         w2T[:, f, :], start=(f == 0), stop=False)
                    nc.tensor.matmul(f2_ps, ones1, ser[:, s, :],
                                     start=False, stop=False)
                    # residual += t_sb via identity matmul (PSUM accumulate)
                    nc.tensor.matmul(f2_ps, ident, t_sb[:, c, :],
                                     start=False, stop=True)
                    rstd2, nmr2 = ln_scale_bias(f2_ps, f"2_{st}_{s}_{c}")
                    y = yp.tile([128, D], F32, name=f"y_{st}_{s}_{c}", tag="y")
                    nc.scalar.activation(
                        y, f2_ps, ACT_F.Identity,
                        scale=rstd2[:, 0:1], bias=nmr2[:, 0:1])
                    if has_g2:
                        nc.vector.tensor_mul(y, y, g2b)
                    if has_b2n:
                        nc.vector.tensor_add(y, y, b2nb)
                    nc.gpsimd.dma_start(
                        out=out_d[nb + c * 128:nb + (c + 1) * 128, s, :], in_=y)

        # ---------------- software-pipelined emission ----------------
        # Phase A (DVE-heavy combine) of supertile st is interleaved with
        # phase B (PE-heavy FF) of supertile st-1 so neither engine starves.
        # Each B slot is split: B1 (out-proj + LN1) is emitted one A-tile
        # before B2 (transpose + FF), hiding the LN1 DVE round-trip that
        # would otherwise stall the PE at the transposes.
        rep_ctx = tc.For_i(0, reps, 1) if reps > 1 else None
        if rep_ctx is not None:
            rep_ctx.__enter__()
        B1_AT = {0: 0, 1: 1, 3: 2}     # after A-tile a, emit B1 of slot v
        B2_AT = {2: 0, 4: 1, 6: 2}     # after A-tile a, emit B2 of slot v
        oTs = {}
        tsbs = {}
        for st in range(N_ST + 1):
            if st < N_ST:
                oTs[st] = otp.tile([128, 4, S, ST_SAMP], BF16,
                                   name=f"oT_{st}", tag="oT")
            for a in range(N_A):
                if st < N_ST:
                    emit_a_tile(st, a, oTs[st])
                if st > 0 and a in B1_AT:
                    tsbs[B1_AT[a]] = emit_b1(st - 1, B1_AT[a], oTs[st - 1])
                if st > 0 and a in B2_AT:
                    emit_b2(st - 1, B2_AT[a], tsbs[B2_AT[a]])
                if st == N_ST and a == max(B2_AT):
                    break
        if rep_ctx is not None:
            rep_ctx.__exit__(None, None, None)

    nc.compile()
    return nc


def _host_prep(cand, slot_q, slot_se, in_w, in_b, out_w, out_b,
               g1, b1n, w1, b1f, w2, b2f, g2, b2n, comb_bf16=True):
    import ml_dtypes
    f32 = np.float32
    bf16 = ml_dtypes.bfloat16
    Wq, Wk, Wv = (in_w[:D], in_w[D:2 * D], in_w[2 * D:])
    bq, bk, bv = (in_b[:D], in_b[D:2 * D], in_b[2 * D:])

    qh = (slot_q @ Wq.T + bq).reshape(S, H, HD)          # [3, 8, 64]
    # Qs[h*3+s, d] = (1/sqrt(hd)) * sum_hd qh[s,h,hd] * Wk[h*64+hd, d]
    Qs = np.zeros((24, D), f32)
    Wk_h = Wk.reshape(H, HD, D)
    for h in range(H):
        Qs[h * 3:(h + 1) * 3, :] = (qh[:, h, :] @ Wk_h[h]) / np.sqrt(HD)

    # e' = hd*8 + h  ->  e = (e' % 8)*64 + e' // 8
    ep = np.arange(D)
    perm = (ep % H) * HD + ep // H
    WvT_p = np.ascontiguousarray(Wv[perm].T)             # [d, e']
    owT_p = np.ascontiguousarray(out_w[:, perm].T)       # [d', e]
    ob2 = out_w @ bv + out_b

    xb = (slot_q + ob2[None, :]).astype(f32)             # [3, 512]
    se = (b1n[None, :] + slot_se).astype(f32)            # [3, 512]
    w1se = (se @ w1.T + b1f[None, :]).astype(f32)        # [3, 2*D]

    consts = {
        "qsT": np.ascontiguousarray(Qs.T).astype(bf16),
        "wvT": WvT_p.astype(bf16),
        "owT": owT_p.astype(bf16),
        "w1T": np.ascontiguousarray(w1.T).astype(bf16),
        "w2T": np.ascontiguousarray(w2.T).astype(bf16),
        "xbr": xb.reshape(1, S * D).astype(bf16),
        "ser": (se + b2f[None, :]).reshape(1, S * D).astype(bf16),
        "w1se": np.ascontiguousarray(w1se.T),            # [2*D, 3] f32
        "one": np.ones((1, 128), bf16),
    }
    flags = (not np.allclose(g1, 1.0), not np.allclose(g2, 1.0),
             not np.allclose(b2n, 0.0))
    if flags[0]:
        consts["g1v"] = g1.astype(f32)
    if flags[1]:
        consts["g2v"] = g2.astype(f32)
    if flags[2]:
        consts["b2nv"] = b2n.astype(f32)

    kvT = np.ascontiguousarray(
        cand.reshape(B, T * K, D).transpose(0, 2, 1)).astype(bf16)  # [8,512,14336]
    return kvT, consts, flags


COMB_BF16 = True


def kernel(**inputs):
    kvT, consts, flags = _host_prep(**inputs, comb_bf16=COMB_BF16)
    key = flags + (COMB_BF16,)
    if key not in _CACHE:
        _CACHE[key] = _build(*flags, comb_bf16=COMB_BF16)
    nc = _CACHE[key]
    in_maps = [dict(consts, kvT=kvT[c]) for c in range(NCORES)]
    res = run_bass_kernel_spmd(nc, in_maps, list(range(NCORES)))
    out = np.concatenate([res.results[c]["out"] for c in range(NCORES)], axis=0)
    return out.astype(np.float32)


if __name__ == "__main__":
    import reference
    import jax as _jax
    with _jax.default_device(_jax.devices("cpu")[0]):
        ins = {k: np.asarray(v) for k, v in reference.setup_inputs().items()}
        exp = np.asarray(reference.reference(**ins))
    got = kernel(**ins)
    rel = np.sqrt(((got - exp) ** 2).mean() / ((exp ** 2).mean() + 1e-30))
    print("shape", got.shape, "rms rel err:", rel)
